# revision 1
# baseline (speedup 1.0000x reference)
"""Conv3d (8,32,48,48,48) * (64,32,3,3,3) -> (8,64,46,46,46), valid, stride 1.

Strategy: data-parallel over batch (1 image per NeuronCore, 8 cores).
Per core the conv is an implicit GEMM:
  out[co, d, h', w'] = sum_{kd,ci,kh,kw} W[co,ci,kd,kh,kw] * X[ci, d+kd, h'+kh, w'+kw]
- contraction K = (kd, ci) = 3*32 = 96 partitions: a SBUF "window" tile
  holds input planes d..d+2 stacked on partitions (plane-major),
- kh, kw are pure free-dim offsets into the window tile (rows step 48),
  so each output tile is 9 accumulating matmuls into one PSUM bank,
- two output planes are processed per window iteration,
- PSUM -> SBUF with fused bias on ScalarE (even plane) / VectorE (odd),
- one contiguous DMA per output plane back to HBM.

Two dtype modes (CONV_MM_DT env):
- "f32r" (default): fp32 operand bits rounded to 11 mantissa bits
  (TF32-like, ~1e-4 rel err). Streams at 1 cycle/row, but the ISA
  requires dst partition base 0, so the two planes' matmuls serialize.
- "bf16": both planes run concurrently on the PE via column tiling
  (col groups 0..1 vs 2..3) -> ~2x matmul throughput, ~3e-3 rel err.
"""

import functools
import os

import numpy as np

import concourse.bacc as bacc
import concourse.tile as tile
from concourse import mybir
from concourse.bass_utils import run_bass_kernel_spmd

# Problem constants (hardcoded per harness contract)
B = 8
CI = 32
DIN = 48
CO = 64
K = 3
DOUT = DIN - K + 1  # 46
SPP = DOUT * DOUT  # 2116 spatial positions per output plane
PLANE = DIN * DIN  # 2304 elements per (ci, plane)
KP = K * CI  # 96 contraction partitions

# h'-row chunking of a 46x46 output plane into PSUM-bank-sized matmuls
CHUNKS = [(0, 10), (10, 9), (19, 9), (28, 9), (37, 9)]  # (h0, rows) -> N = rows*46

F32 = mybir.dt.float32

MODE = os.environ.get("CONV_MM_DT", "fp16")
if MODE == "bf16":
    MM_DT = mybir.dt.bfloat16
    COL_TILE = True
elif MODE == "fp16":
    MM_DT = mybir.dt.float16
    COL_TILE = True
elif MODE == "f32r":
    MM_DT = mybir.dt.float32r
    COL_TILE = False  # fp32r matmul dst must start at partition 0
else:  # plain f32, exact but 4 cycles/row
    MM_DT = F32
    COL_TILE = False


def _pack_mm(a):
    """Host-side cast of a float32 array to the matmul operand format."""
    if MODE == "bf16":
        import ml_dtypes

        return np.ascontiguousarray(a.astype(ml_dtypes.bfloat16))
    if MODE == "fp16":
        return np.ascontiguousarray(a.astype(np.float16))
    if MODE == "f32r":
        # round to 11 mantissa bits, matching walrus's fp32_to_fp32r
        u = np.ascontiguousarray(a, dtype=np.float32).view(np.uint32)
        r = ((u.astype(np.uint64) + 0x800) & 0xFFFFF000).astype(np.uint32)
        return r.view(np.float32)
    return np.ascontiguousarray(a, dtype=np.float32)


@functools.lru_cache(maxsize=1)
def build_program():
    nc = bacc.Bacc("TRN2", target_bir_lowering=False, debug=False)

    x = nc.dram_tensor("x", [DIN * CI, PLANE], MM_DT, kind="ExternalInput").ap()
    wt = nc.dram_tensor("wt", [KP, 9 * CO], MM_DT, kind="ExternalInput").ap()
    b2 = nc.dram_tensor("b2", [2 * CO, 1], F32, kind="ExternalInput").ap()
    y = nc.dram_tensor("y", [CO, DOUT * SPP], F32, kind="ExternalOutput").ap()

    pb_base = CO if COL_TILE else 0
    pb_pos = (0, CO) if COL_TILE else (0, 0)

    with tile.TileContext(nc) as tc:
        with (
            tc.tile_pool(name="wp", bufs=1) as wpool,
            tc.tile_pool(name="xp", bufs=3) as xpool,
            tc.tile_pool(name="op", bufs=3) as opool,
            tc.tile_pool(name="pa", bufs=3, space="PSUM") as papool,
            tc.tile_pool(name="pb", bufs=3, space="PSUM") as pbpool,
            tc.tile_pool(name="ps", bufs=1, space="PSUM") as pspool,
        ):
            wa = wpool.tile([KP, 9 * CO], MM_DT)
            nc.sync.dma_start(wa[:, :], wt)
            bias_t = wpool.tile([2 * CO, 1], F32)
            nc.sync.dma_start(bias_t[:, :], b2)

            # Never-read scratch PSUM bank for "wait absorber" dummy matmuls.
            # The self-loading fp32r matmul has a single sync-wait slot in its
            # LDWEIGHTS uop, so each real matmul may carry at most ONE
            # semaphore wait. Dummies absorb the DMA-completion waits (one
            # dummy per freshly loaded tile) so real matmuls only ever wait on
            # their PSUM slot release.
            scr = pspool.tile([CO, 512], F32)

            # absorb the weights-DMA wait
            nc.tensor.matmul(
                scr[:, :2], wa[:, :CO], wa[:, :2], start=True, stop=True,
                tile_position=(0, 0),
            )

            for t in range(DOUT // 2):
                d0, d1 = 2 * t, 2 * t + 1
                # window tiles: planes d..d+2 stacked on partitions (kd,ci)
                xe = xpool.tile([KP, PLANE], MM_DT, tag="xe", name=f"xe{t}")
                nc.sync.dma_start(xe[:, :], x[CI * d0 : CI * d0 + KP, :])
                xo = xpool.tile([KP, PLANE], MM_DT, tag="xo", name=f"xo{t}")
                nc.sync.dma_start(xo[:, :], x[CI * d1 : CI * d1 + KP, :])
                xe3 = xe[:, :].rearrange("p (h w) -> p h w", w=DIN)
                xo3 = xo[:, :].rearrange("p (h w) -> p h w", w=DIN)

                # absorb the window-DMA waits (see scratch-bank comment above)
                nc.tensor.matmul(
                    scr[:, :2], wa[:, :CO], xe[:, :2], start=True, stop=True,
                    tile_position=(0, 0),
                )
                nc.tensor.matmul(
                    scr[:, :2], wa[:, :CO], xo[:, :2], start=True, stop=True,
                    tile_position=(0, 0),
                )

                ot = opool.tile([2 * CO, SPP], F32)

                for h0, rows in CHUNKS:
                    n = rows * DOUT
                    pa_t = papool.tile([CO, 512], F32, tag="pa")
                    pb_t = pbpool.tile([2 * CO, 512], F32, tag="pb")
                    pa = pa_t[:, :n]
                    pb = pb_t[pb_base : pb_base + CO, :n]
                    for kh in range(K):
                        for kw in range(K):
                            first = kh == 0 and kw == 0
                            last = kh == K - 1 and kw == K - 1
                            lhs = wa[:, (kh * K + kw) * CO : (kh * K + kw + 1) * CO]
                            rhe = xe3[:, h0 + kh : h0 + kh + rows, kw : kw + DOUT]
                            rho = xo3[:, h0 + kh : h0 + kh + rows, kw : kw + DOUT]
                            nc.tensor.matmul(
                                pa[:, :], lhs, rhe, start=first, stop=last,
                                tile_position=(0, 0),
                            )
                            nc.tensor.matmul(
                                pb[:, :], lhs, rho, start=first, stop=last,
                                tile_position=pb_pos,
                            )
                    cs = slice(h0 * DOUT, h0 * DOUT + n)
                    nc.scalar.activation(
                        ot[:CO, cs], pa[:, :],
                        mybir.ActivationFunctionType.Identity,
                        bias=bias_t[:CO, :],
                    )
                    nc.vector.tensor_scalar_add(ot[CO:, cs], pb[:, :], bias_t[CO:, :])
                    # store each chunk as soon as its bias-add lands: keeps the
                    # output DMA overlapped and shortens the kernel tail
                    nc.sync.dma_start(
                        y[:, d0 * SPP + h0 * DOUT : d0 * SPP + h0 * DOUT + n],
                        ot[:CO, cs],
                    )
                    nc.sync.dma_start(
                        y[:, d1 * SPP + h0 * DOUT : d1 * SPP + h0 * DOUT + n],
                        ot[CO:, cs],
                    )

    nc.compile()
    return nc


def make_in_maps(inputs, weight, bias):
    """Host-side shard/pack: returns per-core input maps."""
    inputs = np.ascontiguousarray(np.asarray(inputs, dtype=np.float32))
    weight = np.asarray(weight, dtype=np.float32)
    bias = np.asarray(bias, dtype=np.float32)
    # weights: [(kd,ci), (kh,kw,co)]
    wt = _pack_mm(weight.transpose(2, 1, 3, 4, 0).reshape(KP, 9 * CO))
    b2 = np.ascontiguousarray(np.tile(bias, 2).reshape(2 * CO, 1))
    in_maps = []
    for c in range(B):
        xc = _pack_mm(inputs[c].transpose(1, 0, 2, 3).reshape(DIN * CI, PLANE))
        in_maps.append({"x": xc, "wt": wt, "b2": b2})
    return in_maps


def kernel(inputs, weight, bias, **run_kwargs):
    nc = build_program()
    in_maps = make_in_maps(inputs, weight, bias)
    res = run_bass_kernel_spmd(nc, in_maps, core_ids=list(range(B)), **run_kwargs)
    out = np.stack(
        [res.results[c]["y"].reshape(CO, DOUT, DOUT, DOUT) for c in range(B)]
    )
    return out.astype(np.float32)



# revision 5
# speedup vs baseline: 1.0466x; 1.0466x over previous
"""Conv3d (8,32,48,48,48) * (64,32,3,3,3) -> (8,64,46,46,46), valid, stride 1.

Data-parallel over batch: 1 image per NeuronCore, 8 cores. Per core the conv
is an implicit GEMM with a combined weight matrix so ONE matmul computes TWO
output planes:
  - the window tile holds 4 input planes d0..d0+3 stacked on 128 partitions
    as (kd, ci),
  - lhsT [128, 128]: cols 0-63 = plane d0's weights on rows 0-95 (kd 0-2),
    cols 64-127 = plane d0+1's weights on rows 32-127 (same values shifted
    one 32-row block down), zeros elsewhere,
  - kh, kw are free-dim offsets into the window tile, so each (kh,kw) tap is
    one accumulating K=128 x M=128 matmul; 9 taps complete a PSUM chunk,
  - tap-outer loop: each tap's weights stay loaded across all 5 row chunks
    (9 LDWEIGHTS per plane pair instead of 90),
  - PSUM -> SBUF with fused bias: chunks 0-2 on ScalarE, 3-4 on VectorE,
  - ONE input DMA and ONE output DMA per plane pair (output rows d*64+co are
    contiguous in HBM), keeping the HWDGE trigger queue nearly idle,
  - warmup matmuls on a scratch PSUM bank release the HAM clock gate while
    the first DMAs are in flight.
"""

import functools

import numpy as np

import concourse.bacc as bacc
import concourse.tile as tile
from concourse import mybir
from concourse.bass_utils import run_bass_kernel_spmd

# Problem constants (hardcoded per harness contract)
B = 8
CI = 32
DIN = 48
CO = 64
K = 3
DOUT = DIN - K + 1  # 46
SPP = DOUT * DOUT  # 2116 spatial positions per output plane
PLANE = DIN * DIN  # 2304 elements per (ci, plane)
NTAP = K * K  # 9 (kh, kw) taps per output chunk

# h'-row chunking of a 46x46 output plane into PSUM-bank-sized matmuls
CHUNKS = [(0, 10), (10, 9), (19, 9), (28, 9), (37, 9)]  # (h0, rows) -> N = rows*46

F32 = mybir.dt.float32
F16 = mybir.dt.float16

WARMUP = 20  # scratch matmuls issued before the first real work


@functools.lru_cache(maxsize=1)
def build_program():
    nc = bacc.Bacc("TRN2", target_bir_lowering=False, debug=False)

    x = nc.dram_tensor("x", [DIN * CI, PLANE], F16, kind="ExternalInput").ap()
    wt = nc.dram_tensor("wt", [128, NTAP * 128], F16, kind="ExternalInput").ap()
    b2 = nc.dram_tensor("b2", [2 * CO, 1], F32, kind="ExternalInput").ap()
    # output rows are d*64 + co so each plane pair is one contiguous DMA
    y = nc.dram_tensor("y", [DOUT * CO, SPP], F32, kind="ExternalOutput").ap()

    with tile.TileContext(nc) as tc:
        with (
            tc.tile_pool(name="wp", bufs=1) as wpool,
            tc.tile_pool(name="xp", bufs=3) as xpool,
            tc.tile_pool(name="op", bufs=3) as opool,
            tc.tile_pool(name="p0", bufs=2, space="PSUM") as p0pool,
            tc.tile_pool(name="pa", bufs=1, space="PSUM") as papool,
            tc.tile_pool(name="ps", bufs=1, space="PSUM") as pspool,
        ):
            # Scratch PSUM bank: warmup target + "wait absorber" dummy matmuls
            # (absorbs DMA-completion waits so real matmuls only wait on their
            # PSUM slot).
            scr = pspool.tile([128, 512], F32)

            # Warm up the PE so the HAM clock gate is released (needs ~3.4us
            # of sustained activity) while the first DMAs land.
            wu = wpool.tile([128, 256], F16)
            nc.gpsimd.memset(wu[:, :], 0.0)
            for _ in range(WARMUP):
                nc.tensor.matmul(
                    scr[:, :256], wu[:, :128], wu[:, :256], start=True, stop=True
                )

            wa = wpool.tile([128, NTAP * 128], F16)
            nc.sync.dma_start(wa[:, :], wt)
            bias_t = wpool.tile([2 * CO, 1], F32)
            nc.sync.dma_start(bias_t[:, :], b2)
            # absorb the weights-DMA wait
            nc.tensor.matmul(scr[:, :2], wa[:, :128], wa[:, :2], start=True, stop=True)

            for t in range(DOUT // 2):
                d0 = 2 * t
                # window tile: planes d0..d0+3 stacked on partitions (kd,ci)
                xw = xpool.tile([128, PLANE], F16, tag="xw", name=f"xw{t}")
                nc.sync.dma_start(xw[:, :], x[CI * d0 : CI * d0 + 128, :])
                xw3 = xw[:, :].rearrange("p (h w) -> p h w", w=DIN)

                # absorb the window-DMA wait
                nc.tensor.matmul(
                    scr[:, :2], wa[:, :128], xw[:, :2], start=True, stop=True
                )

                pt = [
                    (p0pool if c < 2 else papool).tile(
                        [128, 512], F32, tag=f"pc{c}", name=f"pc{c}_{t}"
                    )
                    for c in range(5)
                ]
                ot = opool.tile([128, SPP], F32)

                for tap in range(NTAP):
                    kh, kw = divmod(tap, K)
                    lhs = wa[:, tap * 128 : (tap + 1) * 128]
                    for c, (h0, rows) in enumerate(CHUNKS):
                        n = rows * DOUT
                        nc.tensor.matmul(
                            pt[c][:, :n],
                            lhs,
                            xw3[:, h0 + kh : h0 + kh + rows, kw : kw + DOUT],
                            start=(tap == 0),
                            stop=(tap == NTAP - 1),
                        )

                for c, (h0, rows) in enumerate(CHUNKS):
                    n = rows * DOUT
                    cs = slice(h0 * DOUT, h0 * DOUT + n)
                    if c < 3:
                        nc.scalar.activation(
                            ot[:, cs],
                            pt[c][:, :n],
                            mybir.ActivationFunctionType.Identity,
                            bias=bias_t[:, :],
                        )
                    else:
                        nc.vector.tensor_scalar_add(
                            ot[:, cs], pt[c][:, :n], bias_t[:, :]
                        )

                nc.sync.dma_start(y[CO * d0 : CO * d0 + 2 * CO, :], ot[:, :])

    nc.compile()
    return nc


def _f16(a):
    return np.ascontiguousarray(np.asarray(a, dtype=np.float32).astype(np.float16))


def make_in_maps(inputs, weight, bias):
    """Host-side shard/pack: returns per-core input maps."""
    inputs = np.ascontiguousarray(np.asarray(inputs, dtype=np.float32))
    weight = np.asarray(weight, dtype=np.float32)
    bias = np.asarray(bias, dtype=np.float32)

    # combined weights: [p=(kd,ci), tap*128 + (plane, co)]
    wt5 = weight.transpose(2, 1, 3, 4, 0)  # [kd, ci, kh, kw, co]
    wcomb = np.zeros((128, NTAP, 128), np.float32)
    for kh in range(K):
        for kw in range(K):
            tap = kh * K + kw
            blk = wt5[:, :, kh, kw, :].reshape(K * CI, CO)  # [(kd ci), co]
            wcomb[0:96, tap, 0:64] = blk  # plane d0: kd 0-2 on rows 0-95
            wcomb[32:128, tap, 64:128] = blk  # plane d1: kd 0-2 on rows 32-127
    wtp = _f16(wcomb.reshape(128, NTAP * 128))

    b2 = np.ascontiguousarray(np.tile(bias, 2).reshape(2 * CO, 1))
    in_maps = []
    for c in range(B):
        xc = _f16(inputs[c].transpose(1, 0, 2, 3).reshape(DIN * CI, PLANE))
        in_maps.append({"x": xc, "wt": wtp, "b2": b2})
    return in_maps


def kernel(inputs, weight, bias, **run_kwargs):
    nc = build_program()
    in_maps = make_in_maps(inputs, weight, bias)
    res = run_bass_kernel_spmd(nc, in_maps, core_ids=list(range(B)), **run_kwargs)
    out = np.stack(
        [
            res.results[c]["y"]
            .reshape(DOUT, CO, SPP)
            .transpose(1, 0, 2)
            .reshape(CO, DOUT, DOUT, DOUT)
            for c in range(B)
        ]
    )
    return out.astype(np.float32)
